# revision 2
# baseline (speedup 1.0000x reference)
"""Adstock transform on 8 trn2 cores — J=8 polyphase decimation, 3-engine split.

r[b, t, c] = x[b, t, c] + d[c] * r[b, t-1, c],  d = sigmoid(decay)

The DVE scan op runs at ~2 cyc/elem (feedback-limited), so a direct scan costs
~137us/core.  Instead, de-interleave time into 8 phases (host-side permute):
  t = 8k + p,  phase arrays of length K = T/8 = 1024 per batch.
Build the 8-step block sums z8[k] = sum_{j<8} d^j x[8k+7-j] with a tree of
(scale, add) passes, scan only z8 (T/8 elements, decay d^8) -> R[k] = r[8k+7],
then reconstruct phases 0-6 with one (scale, add) each:
  r_ph = partial + d^j * carrier   (carrier = R[k-1] or an earlier phase).

Per-partition scales run on ScalarE (activation Copy, 1 cyc/elem) and DVE
tensor_scalar (4x mode, 0.25 cyc/elem); adds run on DVE tensor_tensor (2x,
0.5 cyc/elem) and GpSimd tensor_tensor; the only scan left is T/8 long.
DVE ends up ~0.9 cyc/elem instead of 2 -> the kernel is DMA-bound.

Layout: host permutes x to phase-major c-rows [b_loc, C, 8*K] bf16
(x_perm[b, c, p*K + k] = x[b, 8k+p, c]), one 2 MiB load + ~2 MiB of stores
per batch, all contiguous 16KB/14KB/2KB partition lines.  bf16 I/O halves
HBM traffic; measured end-to-end rel err ~1e-2 vs the 2e-2 gate.
"""

import numpy as np
import ml_dtypes

import concourse.bacc as bacc
import concourse.mybir as mybir
from concourse.bass_utils import run_bass_kernel_spmd
from concourse.tile import TileContext

F32 = mybir.dt.float32
BF16 = mybir.dt.bfloat16
_BF16_NP = ml_dtypes.bfloat16

B, T, C = 64, 8192, 128
NCORES = 8
B_LOC = B // NCORES  # 8 batches per core
J = 8                # decimation factor (phases)
K = T // J           # 1024 steps per phase array


def build_nc():
    nc = bacc.Bacc("TRN2", target_bir_lowering=False, debug=False)
    x = nc.dram_tensor("x", [B_LOC, C, T], BF16, kind="ExternalInput").ap()
    dpow = nc.dram_tensor("dpow", [C, 4], F32, kind="ExternalInput").ap()
    y = nc.dram_tensor("y", [B_LOC, C, T], BF16, kind="ExternalOutput").ap()

    M = mybir.AluOpType
    AF = mybir.ActivationFunctionType

    with TileContext(nc) as tc:
        with (
            tc.tile_pool(name="const", bufs=1) as cpool,
            tc.tile_pool(name="inp", bufs=3) as inp,
            tc.tile_pool(name="outp", bufs=3) as outp,
            tc.tile_pool(name="rp", bufs=3) as rp,
            tc.tile_pool(name="sp", bufs=2) as sp,
            tc.tile_pool(name="tp", bufs=2) as tp,
        ):
            dp = cpool.tile([C, 4], F32)
            nc.sync.dma_start(out=dp, in_=dpow)
            d1 = dp[:, 0:1]
            d2 = dp[:, 1:2]
            d4 = dp[:, 2:3]
            d8 = dp[:, 3:4]
            d8_bc = d8.broadcast_to([C, K])

            def sc(tag, dcol, src):
                """ScalarE per-partition scale: out = dcol * src."""
                t = tp.tile([C, K], BF16, tag=tag)
                nc.scalar.activation(out=t, in_=src, func=AF.Copy, scale=dcol)
                return t

            def ts(tag, dcol, src):
                """DVE tensor_scalar per-partition scale (4x mode)."""
                t = tp.tile([C, K], BF16, tag=tag)
                nc.vector.tensor_scalar(
                    out=t, in0=src, scalar1=dcol, scalar2=None, op0=M.mult
                )
                return t

            for b in range(B_LOC):
                ld = inp.tile([C, T], BF16, tag="in")
                nc.sync.dma_start(out=ld, in_=x[b])
                ph = [ld[:, p * K : (p + 1) * K] for p in range(J)]

                ost = outp.tile([C, 7 * K], BF16, tag="out")
                rt = rp.tile([C, K + 1], BF16, tag="r")

                # ---- prep tree: z8[k] = sum_{j<8} d^j x[8k+7-j] ----
                t0 = sc("t0", d1, ph[0])
                s1_01 = sp.tile([C, K], BF16, tag="s1_01")
                nc.vector.tensor_tensor(out=s1_01, in0=t0, in1=ph[1], op=M.add)

                t1 = sc("t1", d1, ph[2])
                s1_23 = sp.tile([C, K], BF16, tag="s1_23")
                nc.gpsimd.tensor_tensor(out=s1_23, in0=t1, in1=ph[3], op=M.add)

                t2 = sc("t2", d1, ph[4])
                s1_45 = sp.tile([C, K], BF16, tag="s1_45")
                nc.vector.tensor_tensor(out=s1_45, in0=t2, in1=ph[5], op=M.add)

                t3 = sc("t3", d1, ph[6])
                s1_67 = sp.tile([C, K], BF16, tag="s1_67")
                nc.gpsimd.tensor_tensor(out=s1_67, in0=t3, in1=ph[7], op=M.add)

                u0 = sc("u0", d2, s1_01)
                s2_03 = sp.tile([C, K], BF16, tag="s2_03")
                nc.vector.tensor_tensor(out=s2_03, in0=u0, in1=s1_23, op=M.add)

                u1 = sc("u1", d2, s1_45)
                s2_47 = sp.tile([C, K], BF16, tag="s2_47")
                nc.vector.tensor_tensor(out=s2_47, in0=u1, in1=s1_67, op=M.add)

                v0 = sc("v0", d4, s2_03)
                z8 = sp.tile([C, K], BF16, tag="z8")
                nc.vector.tensor_tensor(out=z8, in0=v0, in1=s2_47, op=M.add)

                # ---- block scan: R[k] = d^8 R[k-1] + z8[k] = r[8k+7] ----
                nc.vector.memset(rt[:, 0:1], 0.0)
                nc.vector.tensor_tensor_scan(
                    out=rt[:, 1 : K + 1],
                    data0=d8_bc,
                    data1=z8,
                    initial=0.0,
                    op0=M.mult,
                    op1=M.add,
                )
                S = rt[:, 0:K]  # R[k-1], with R[-1] = 0

                # ---- reconstruction: phases 0-6 (phase 7 = scan output) ----
                a0 = sc("a0", d1, S)
                nc.vector.tensor_tensor(
                    out=ost[:, 0:K], in0=a0, in1=ph[0], op=M.add
                )
                a1 = sc("a1", d2, S)
                p1 = ost[:, K : 2 * K]
                nc.vector.tensor_tensor(out=p1, in0=a1, in1=s1_01, op=M.add)

                b1 = ts("b1", d1, p1)
                nc.gpsimd.tensor_tensor(
                    out=ost[:, 2 * K : 3 * K], in0=b1, in1=ph[2], op=M.add
                )

                a3 = ts("a3", d4, S)
                p3 = ost[:, 3 * K : 4 * K]
                nc.vector.tensor_tensor(out=p3, in0=a3, in1=s2_03, op=M.add)

                b3 = ts("b3", d1, p3)
                nc.gpsimd.tensor_tensor(
                    out=ost[:, 4 * K : 5 * K], in0=b3, in1=ph[4], op=M.add
                )

                c5 = sc("c5", d2, p3)
                p5 = ost[:, 5 * K : 6 * K]
                nc.vector.tensor_tensor(out=p5, in0=c5, in1=s1_45, op=M.add)

                b5 = ts("b5", d1, p5)
                nc.vector.tensor_tensor(
                    out=ost[:, 6 * K : 7 * K], in0=b5, in1=ph[6], op=M.add
                )

                # ---- stores: phases 0-6 from ost, phase 7 from scan out ----
                nc.scalar.dma_start(out=y[b, :, 0 : 7 * K], in_=ost)
                nc.scalar.dma_start(out=y[b, :, 7 * K : T], in_=rt[:, 1 : K + 1])
    nc.finalize()
    return nc


_NC_CACHE = {}


def _get_nc():
    if "nc" not in _NC_CACHE:
        _NC_CACHE["nc"] = build_nc()
    return _NC_CACHE["nc"]


def _make_dpow(decay: np.ndarray) -> np.ndarray:
    d = 1.0 / (1.0 + np.exp(-decay.astype(np.float64)))  # [C]
    dp = np.stack([d, d**2, d**4, d**8], axis=1)  # [C, 4]
    return dp.astype(np.float32).copy()


def _permute_in(xc: np.ndarray) -> np.ndarray:
    """[b_loc, T, C] f32 -> phase-major [b_loc, C, T] bf16."""
    xp = xc.reshape(B_LOC, K, J, C).transpose(0, 3, 2, 1)  # [b, c, p, k]
    return np.ascontiguousarray(xp).reshape(B_LOC, C, T).astype(_BF16_NP)


def _unpermute_out(yp: np.ndarray) -> np.ndarray:
    """phase-major [b_loc, C, T] bf16 -> [b_loc, T, C] f32."""
    ya = np.asarray(yp).astype(np.float32).reshape(B_LOC, C, J, K)
    return np.ascontiguousarray(ya.transpose(0, 3, 2, 1)).reshape(B_LOC, T, C)


def make_in_maps(x, decay):
    x = np.asarray(x, dtype=np.float32)
    dp = _make_dpow(np.asarray(decay))
    return [
        {"x": _permute_in(x[i * B_LOC : (i + 1) * B_LOC]), "dpow": dp}
        for i in range(NCORES)
    ]


def run(x, decay, trace=False, tmpdir=None, trace_cores=None):
    nc = _get_nc()
    in_maps = make_in_maps(x, decay)
    res = run_bass_kernel_spmd(
        nc,
        in_maps,
        list(range(NCORES)),
        trace=trace,
        tmpdir=tmpdir,
        trace_cores=trace_cores,
    )
    out = np.concatenate([_unpermute_out(r["y"]) for r in res.results], axis=0)
    return out, res


def kernel(x: np.ndarray, decay: np.ndarray) -> np.ndarray:
    out, _ = run(x, decay)
    return out


# revision 8
# speedup vs baseline: 1.0817x; 1.0817x over previous
"""Adstock transform on 8 trn2 cores — J=8 polyphase decimation, DVE+ScalarE.

r[b, t, c] = x[b, t, c] + d[c] * r[b, t-1, c],  d = sigmoid(decay)

The DVE scan op runs at ~2 cyc/elem (feedback-limited), so a direct scan costs
~137us/core.  Instead, de-interleave time into 8 phases (host-side permute):
  t = 8k + p,  phase arrays of length K = T/8 = 1024 per batch.
Build the 8-step block sums z8[k] = sum_{j<8} d^j x[8k+7-j] with a tree of
(scale, add) passes, scan only z8 (T/8 elements, decay d^8) -> R[k] = r[8k+7],
then reconstruct phases 0-6 with one (scale, add) each:
  r_ph = partial + d^j * carrier   (carrier = R[k-1] or an earlier phase).

Engine split: per-partition scales on ScalarE (activation Copy, ~0.85 ns/elem)
and DVE tensor_scalar (4x mode) for the latency-critical chain scales; adds on
DVE tensor_tensor (2x mode).  GpSimd is intentionally NOT used: its SBUF
traffic was measured to slow concurrent DVE ops ~2.4x (contention), costing
more than it offloads.  Batches are processed in fused pairs so elementwise
ops run at FD=2048 and DMAs move 4 MiB slabs.

Layout: host permutes x to phase-major c-rows [4, C, 16384] bf16 per core
(x[i, c, p*2048 + j*1024 + k] = x_orig[2i+j, 8k+p, c]); bf16 I/O halves HBM
traffic (measured end-to-end rel err ~1e-2 vs the 2e-2 gate).
"""

import numpy as np
import ml_dtypes

import concourse.bacc as bacc
import concourse.mybir as mybir
from concourse.bass_utils import run_bass_kernel_spmd
from concourse.tile import TileContext

F32 = mybir.dt.float32
BF16 = mybir.dt.bfloat16
_BF16_NP = ml_dtypes.bfloat16

B, T, C = 64, 8192, 128
NCORES = 8
B_LOC = B // NCORES  # 8 batches per core
J = 8                # decimation factor (phases)
K = T // J           # 1024 scan steps per phase per batch
P = 2                # batches fused per pair
NP = B_LOC // P      # 4 pairs per core
F = P * K            # 2048: fused elementwise op width
TP = P * T           # 16384: free size of one pair slab


def build_nc():
    nc = bacc.Bacc("TRN2", target_bir_lowering=False, debug=False)
    x = nc.dram_tensor("x", [NP, C, TP], BF16, kind="ExternalInput").ap()
    dpow = nc.dram_tensor("dpow", [C, 4], F32, kind="ExternalInput").ap()
    y = nc.dram_tensor("y", [NP, C, TP], BF16, kind="ExternalOutput").ap()

    M = mybir.AluOpType
    AF = mybir.ActivationFunctionType

    with TileContext(nc) as tc:
        with (
            tc.tile_pool(name="const", bufs=1) as cpool,
            tc.tile_pool(name="inp", bufs=2) as inp,
            tc.tile_pool(name="outp", bufs=10) as outp,
            tc.tile_pool(name="rp", bufs=3) as rp,
            tc.tile_pool(name="sp", bufs=2) as sp,
            tc.tile_pool(name="tp", bufs=2) as tp,
        ):
            dp = cpool.tile([C, 4], F32)
            nc.sync.dma_start(out=dp, in_=dpow)
            d1 = dp[:, 0:1]
            d2 = dp[:, 1:2]
            d4 = dp[:, 2:3]
            d8 = dp[:, 3:4]
            d8_bc = d8.broadcast_to([C, K])

            for i in range(NP):
                # split loads: phases 0-3, then 4-7 (compute starts sooner)
                ld = inp.tile([C, TP], BF16, tag="in")
                nc.sync.dma_start(out=ld[:, 0 : 4 * F], in_=x[i, :, 0 : 4 * F])
                nc.sync.dma_start(out=ld[:, 4 * F : TP], in_=x[i, :, 4 * F : TP])
                # phase slice (both batches of the pair): FD=2048
                xp = [ld[:, p * F : (p + 1) * F] for p in range(J)]
                # phase slice of one batch j: FD=1024
                xpj = [
                    [ld[:, p * F + j * K : p * F + (j + 1) * K] for j in range(P)]
                    for p in range(J)
                ]

                rt = rp.tile([C, 2 * K + 2], BF16, tag="r")

                def fma(tag, dcol, src, addend, fd=F):
                    """tile = dcol*src (ScalarE) ; tile += addend (DVE)."""
                    t = sp.tile([C, fd], BF16, tag=tag)
                    nc.scalar.activation(out=t, in_=src, func=AF.Copy, scale=dcol)
                    nc.vector.tensor_tensor(out=t, in0=t, in1=addend, op=M.add)
                    return t

                # ---- prep tree: z8[k] = sum_{j<8} d^j x[8k+7-j] ----
                s1_01 = fma("s1_01", d1, xp[0], xp[1])
                s1_23 = fma("s1_23", d1, xp[2], xp[3])
                s1_45 = fma("s1_45", d1, xp[4], xp[5])
                s1_67 = fma("s1_67", d1, xp[6], xp[7])
                s2_03 = fma("s2_03", d2, s1_01, s1_23)
                s2_47 = fma("s2_47", d2, s1_45, s1_67)
                z8 = fma("z8", d4, s2_03, s2_47)

                # ---- per-batch block scans: R[k] = d^8 R[k-1] + z8[k] ----
                # rt columns: [0]=0-pad | [1..K]=R_j0 | [K+1]=0-pad | [K+2..2K+1]=R_j1
                nc.vector.memset(rt[:, 0:1], 0.0)
                nc.vector.memset(rt[:, K + 1 : K + 2], 0.0)
                S = [rt[:, 0:K], rt[:, K + 1 : 2 * K + 1]]
                R7 = [rt[:, 1 : K + 1], rt[:, K + 2 : 2 * K + 2]]
                for j in range(P):
                    nc.vector.tensor_tensor_scan(
                        out=R7[j],
                        data0=d8_bc,
                        data1=z8[:, j * K : (j + 1) * K],
                        initial=0.0,
                        op0=M.mult,
                        op1=M.add,
                    )
                # phase-7 stores issued immediately so rt recycles early
                nc.scalar.dma_start(out=y[i, :, 7 * F : 7 * F + K], in_=R7[0])
                nc.scalar.dma_start(out=y[i, :, 7 * F + K : TP], in_=R7[1])

                # ---- reconstruction into per-phase tiles, eager stores ----
                ph_t = {
                    p: outp.tile([C, F], BF16, tag="pho", name=f"pho_{i}_{p}")
                    for p in range(7)
                }

                def store(p):
                    nc.scalar.dma_start(
                        out=y[i, :, p * F : (p + 1) * F], in_=ph_t[p]
                    )

                # S-based phases, per batch j (FD=1024)
                for j in range(P):
                    js = slice(j * K, (j + 1) * K)
                    a0 = tp.tile([C, K], BF16, tag="a0")
                    nc.scalar.activation(out=a0, in_=S[j], func=AF.Copy, scale=d1)
                    nc.vector.tensor_tensor(
                        out=ph_t[0][:, js], in0=a0, in1=xpj[0][j], op=M.add
                    )
                    a1 = tp.tile([C, K], BF16, tag="a1")
                    nc.scalar.activation(out=a1, in_=S[j], func=AF.Copy, scale=d2)
                    nc.vector.tensor_tensor(
                        out=ph_t[1][:, js], in0=a1, in1=s1_01[:, js], op=M.add
                    )
                    a3 = tp.tile([C, K], BF16, tag="a3")
                    nc.vector.tensor_scalar(
                        out=a3, in0=S[j], scalar1=d4, scalar2=None, op0=M.mult
                    )
                    nc.vector.tensor_tensor(
                        out=ph_t[3][:, js], in0=a3, in1=s2_03[:, js], op=M.add
                    )
                store(0)
                store(1)
                store(3)

                # chained phases (FD=2048), chain scales on DVE TS (4x)
                def chain(tag, dcol, src, addend, p_dst):
                    t = tp.tile([C, F], BF16, tag=tag)
                    nc.vector.tensor_scalar(
                        out=t, in0=src, scalar1=dcol, scalar2=None, op0=M.mult
                    )
                    nc.vector.tensor_tensor(
                        out=ph_t[p_dst], in0=t, in1=addend, op=M.add
                    )
                    store(p_dst)

                chain("ch0", d1, ph_t[1], xp[2], 2)
                chain("ch1", d1, ph_t[3], xp[4], 4)
                chain("ch0", d2, ph_t[3], s1_45, 5)
                chain("ch1", d1, ph_t[5], xp[6], 6)
    nc.finalize()
    return nc


_NC_CACHE = {}


def _get_nc():
    if "nc" not in _NC_CACHE:
        _NC_CACHE["nc"] = build_nc()
    return _NC_CACHE["nc"]


def _make_dpow(decay: np.ndarray) -> np.ndarray:
    d = 1.0 / (1.0 + np.exp(-decay.astype(np.float64)))  # [C]
    dp = np.stack([d, d**2, d**4, d**8], axis=1)  # [C, 4]
    return dp.astype(np.float32).copy()


def _permute_in(xc: np.ndarray) -> np.ndarray:
    """[b_loc, T, C] f32 -> pair-fused phase-major [NP, C, TP] bf16."""
    xp = xc.reshape(NP, P, K, J, C).transpose(0, 4, 3, 1, 2)  # [i, c, p, j, k]
    return np.ascontiguousarray(xp).reshape(NP, C, TP).astype(_BF16_NP)


def _unpermute_out(yp: np.ndarray) -> np.ndarray:
    """pair-fused phase-major [NP, C, TP] bf16 -> [b_loc, T, C] f32."""
    ya = np.asarray(yp).astype(np.float32).reshape(NP, C, J, P, K)
    return np.ascontiguousarray(ya.transpose(0, 3, 4, 2, 1)).reshape(B_LOC, T, C)


def make_in_maps(x, decay):
    x = np.asarray(x, dtype=np.float32)
    dp = _make_dpow(np.asarray(decay))
    return [
        {"x": _permute_in(x[i * B_LOC : (i + 1) * B_LOC]), "dpow": dp}
        for i in range(NCORES)
    ]


def run(x, decay, trace=False, tmpdir=None, trace_cores=None):
    nc = _get_nc()
    in_maps = make_in_maps(x, decay)
    res = run_bass_kernel_spmd(
        nc,
        in_maps,
        list(range(NCORES)),
        trace=trace,
        tmpdir=tmpdir,
        trace_cores=trace_cores,
    )
    out = np.concatenate([_unpermute_out(r["y"]) for r in res.results], axis=0)
    return out, res


def kernel(x: np.ndarray, decay: np.ndarray) -> np.ndarray:
    out, _ = run(x, decay)
    return out


# revision 10
# speedup vs baseline: 1.3836x; 1.2791x over previous
"""Adstock transform on 8 trn2 cores — J=8 polyphase decimation, DVE+ScalarE.

r[b, t, c] = x[b, t, c] + d[c] * r[b, t-1, c],  d = sigmoid(decay)

The DVE scan op runs at ~2 cyc/elem (feedback-limited), so a direct scan costs
~137us/core.  Instead, de-interleave time into 8 phases (host-side permute):
  t = 8k + p,  phase arrays of length K = T/8 = 1024 per batch.
Build the 8-step block sums z8[k] = sum_{j<8} d^j x[8k+7-j] with a tree of
(scale, add) passes, scan only z8 (T/8 elements, decay d^8) -> R[k] = r[8k+7],
then reconstruct phases 0-6 with one (scale, add) each:
  r_ph = partial + d^j * carrier   (carrier = R[k-1] or an earlier phase).

Engine split: per-partition scales on ScalarE (activation Copy, ~0.85 ns/elem)
and DVE tensor_scalar (4x mode) for the latency-critical chain scales; adds on
DVE tensor_tensor (2x mode).  GpSimd is intentionally NOT used: its SBUF
traffic was measured to slow concurrent DVE ops ~2.4x (contention), costing
more than it offloads.  Batches are processed in fused pairs so elementwise
ops run at FD=2048 and DMAs move 4 MiB slabs.

Layout: host permutes x to phase-major c-rows [4, C, 16384] bf16 per core
(x[i, c, p*2048 + j*1024 + k] = x_orig[2i+j, 8k+p, c]); bf16 I/O halves HBM
traffic (measured end-to-end rel err ~1e-2 vs the 2e-2 gate).
"""

import numpy as np
import ml_dtypes

import concourse.bacc as bacc
import concourse.mybir as mybir
from concourse.bass_utils import run_bass_kernel_spmd
from concourse.tile import TileContext

F32 = mybir.dt.float32
BF16 = mybir.dt.bfloat16
_BF16_NP = ml_dtypes.bfloat16

B, T, C = 64, 8192, 128
NCORES = 8
B_LOC = B // NCORES  # 8 batches per core
J = 8                # decimation factor (phases)
K = T // J           # 1024 scan steps per phase per batch
P = 2                # batches fused per pair
NP = B_LOC // P      # 4 pairs per core
F = P * K            # 2048: fused elementwise op width
TP = P * T           # 16384: free size of one pair slab


def build_nc():
    nc = bacc.Bacc("TRN2", target_bir_lowering=False, debug=False)
    x = nc.dram_tensor("x", [NP, C, TP], BF16, kind="ExternalInput").ap()
    dpow = nc.dram_tensor("dpow", [C, 4], F32, kind="ExternalInput").ap()
    y = nc.dram_tensor("y", [NP, C, TP], BF16, kind="ExternalOutput").ap()

    M = mybir.AluOpType
    AF = mybir.ActivationFunctionType

    with TileContext(nc) as tc:
        with (
            tc.tile_pool(name="const", bufs=1) as cpool,
            tc.tile_pool(name="inp", bufs=2) as inp,
            tc.tile_pool(name="outp", bufs=10) as outp,
            tc.tile_pool(name="rp", bufs=3) as rp,
            tc.tile_pool(name="sp", bufs=2) as sp,
            tc.tile_pool(name="tp", bufs=2) as tp,
        ):
            dp = cpool.tile([C, 4], F32)
            nc.sync.dma_start(out=dp, in_=dpow)
            d1 = dp[:, 0:1]
            d2 = dp[:, 1:2]
            d4 = dp[:, 2:3]
            d8 = dp[:, 3:4]
            d8_bc = d8.broadcast_to([C, K])

            for i in range(NP):
                # split loads: phases 0-3, then 4-7 (compute starts sooner)
                ld = inp.tile([C, TP], BF16, tag="in")
                nc.sync.dma_start(out=ld[:, 0 : 4 * F], in_=x[i, :, 0 : 4 * F])
                nc.sync.dma_start(out=ld[:, 4 * F : TP], in_=x[i, :, 4 * F : TP])
                # phase slice (both batches of the pair): FD=2048
                xp = [ld[:, p * F : (p + 1) * F] for p in range(J)]
                # phase slice of one batch j: FD=1024
                xpj = [
                    [ld[:, p * F + j * K : p * F + (j + 1) * K] for j in range(P)]
                    for p in range(J)
                ]

                rt = rp.tile([C, 2 * K + 2], BF16, tag="r")

                def fma(tag, dcol, src, addend, fd=F):
                    """tile = dcol*src (ScalarE) ; tile += addend (DVE)."""
                    t = sp.tile([C, fd], BF16, tag=tag)
                    nc.scalar.activation(out=t, in_=src, func=AF.Copy, scale=dcol)
                    nc.vector.tensor_tensor(out=t, in0=t, in1=addend, op=M.add)
                    return t

                # ---- prep tree: z8[k] = sum_{j<8} d^j x[8k+7-j] ----
                s1_01 = fma("s1_01", d1, xp[0], xp[1])
                s1_23 = fma("s1_23", d1, xp[2], xp[3])
                s1_45 = fma("s1_45", d1, xp[4], xp[5])
                s1_67 = fma("s1_67", d1, xp[6], xp[7])
                s2_03 = fma("s2_03", d2, s1_01, s1_23)
                s2_47 = fma("s2_47", d2, s1_45, s1_67)
                z8 = fma("z8", d4, s2_03, s2_47)

                # ---- per-batch block scans: R[k] = d^8 R[k-1] + z8[k] ----
                # rt columns: [0]=0-pad | [1..K]=R_j0 | [K+1]=0-pad | [K+2..2K+1]=R_j1
                nc.vector.memset(rt[:, 0:1], 0.0)
                nc.vector.memset(rt[:, K + 1 : K + 2], 0.0)
                S = [rt[:, 0:K], rt[:, K + 1 : 2 * K + 1]]
                R7 = [rt[:, 1 : K + 1], rt[:, K + 2 : 2 * K + 2]]
                for j in range(P):
                    nc.vector.tensor_tensor_scan(
                        out=R7[j],
                        data0=d8_bc,
                        data1=z8[:, j * K : (j + 1) * K],
                        initial=0.0,
                        op0=M.mult,
                        op1=M.add,
                    )
                # phase-7 stores issued immediately so rt recycles early.
                # All stores ride the (otherwise idle) GpSimd SWDGE queue so
                # they never FIFO-block the next pair's ScalarE/load work.
                nc.gpsimd.dma_start(out=y[i, :, 7 * F : 7 * F + K], in_=R7[0])
                nc.gpsimd.dma_start(out=y[i, :, 7 * F + K : TP], in_=R7[1])

                # ---- reconstruction into per-phase tiles, eager stores ----
                ph_t = {
                    p: outp.tile([C, F], BF16, tag="pho", name=f"pho_{i}_{p}")
                    for p in range(7)
                }

                def store(p):
                    nc.gpsimd.dma_start(
                        out=y[i, :, p * F : (p + 1) * F], in_=ph_t[p]
                    )

                # S-based phases, per batch j (FD=1024); scales on DVE TS so
                # they don't sit behind scan-dependent ops in the ScalarE FIFO
                for j in range(P):
                    js = slice(j * K, (j + 1) * K)
                    a0 = tp.tile([C, K], BF16, tag="a0")
                    nc.vector.tensor_scalar(
                        out=a0, in0=S[j], scalar1=d1, scalar2=None, op0=M.mult
                    )
                    nc.vector.tensor_tensor(
                        out=ph_t[0][:, js], in0=a0, in1=xpj[0][j], op=M.add
                    )
                    a1 = tp.tile([C, K], BF16, tag="a1")
                    nc.vector.tensor_scalar(
                        out=a1, in0=S[j], scalar1=d2, scalar2=None, op0=M.mult
                    )
                    nc.vector.tensor_tensor(
                        out=ph_t[1][:, js], in0=a1, in1=s1_01[:, js], op=M.add
                    )
                    a3 = tp.tile([C, K], BF16, tag="a3")
                    nc.vector.tensor_scalar(
                        out=a3, in0=S[j], scalar1=d4, scalar2=None, op0=M.mult
                    )
                    nc.vector.tensor_tensor(
                        out=ph_t[3][:, js], in0=a3, in1=s2_03[:, js], op=M.add
                    )
                store(0)
                store(1)
                store(3)

                # chained phases (FD=2048), chain scales on DVE TS (4x)
                def chain(tag, dcol, src, addend, p_dst):
                    t = tp.tile([C, F], BF16, tag=tag)
                    nc.vector.tensor_scalar(
                        out=t, in0=src, scalar1=dcol, scalar2=None, op0=M.mult
                    )
                    nc.vector.tensor_tensor(
                        out=ph_t[p_dst], in0=t, in1=addend, op=M.add
                    )
                    store(p_dst)

                chain("ch0", d1, ph_t[1], xp[2], 2)
                chain("ch1", d1, ph_t[3], xp[4], 4)
                chain("ch0", d2, ph_t[3], s1_45, 5)
                chain("ch1", d1, ph_t[5], xp[6], 6)
    nc.finalize()
    return nc


_NC_CACHE = {}


def _get_nc():
    if "nc" not in _NC_CACHE:
        _NC_CACHE["nc"] = build_nc()
    return _NC_CACHE["nc"]


def _make_dpow(decay: np.ndarray) -> np.ndarray:
    d = 1.0 / (1.0 + np.exp(-decay.astype(np.float64)))  # [C]
    dp = np.stack([d, d**2, d**4, d**8], axis=1)  # [C, 4]
    return dp.astype(np.float32).copy()


def _permute_in(xc: np.ndarray) -> np.ndarray:
    """[b_loc, T, C] f32 -> pair-fused phase-major [NP, C, TP] bf16."""
    xp = xc.reshape(NP, P, K, J, C).transpose(0, 4, 3, 1, 2)  # [i, c, p, j, k]
    return np.ascontiguousarray(xp).reshape(NP, C, TP).astype(_BF16_NP)


def _unpermute_out(yp: np.ndarray) -> np.ndarray:
    """pair-fused phase-major [NP, C, TP] bf16 -> [b_loc, T, C] f32."""
    ya = np.asarray(yp).astype(np.float32).reshape(NP, C, J, P, K)
    return np.ascontiguousarray(ya.transpose(0, 3, 4, 2, 1)).reshape(B_LOC, T, C)


def make_in_maps(x, decay):
    x = np.asarray(x, dtype=np.float32)
    dp = _make_dpow(np.asarray(decay))
    return [
        {"x": _permute_in(x[i * B_LOC : (i + 1) * B_LOC]), "dpow": dp}
        for i in range(NCORES)
    ]


def run(x, decay, trace=False, tmpdir=None, trace_cores=None):
    nc = _get_nc()
    in_maps = make_in_maps(x, decay)
    res = run_bass_kernel_spmd(
        nc,
        in_maps,
        list(range(NCORES)),
        trace=trace,
        tmpdir=tmpdir,
        trace_cores=trace_cores,
    )
    out = np.concatenate([_unpermute_out(r["y"]) for r in res.results], axis=0)
    return out, res


def kernel(x: np.ndarray, decay: np.ndarray) -> np.ndarray:
    out, _ = run(x, decay)
    return out
